# revision 16
# baseline (speedup 1.0000x reference)
"""3D Haar DWT (depth-1) Trainium2 kernel.

Full inputs: x [4, 4, 64, 256, 256] f32 + six banded Haar matrices
(hardcoded math: every output element is +-2^-1.5 times a +-sum of a
2x2x2 block). Returns the 8 subbands (LLL, LLH, LHL, LHH, HLL, HLH,
HHL, HHH), each [4, 4, 32, 128, 128] f32.

Sharding: data-parallel over N*C = 16 sample-channels, 2 per core on
8 cores. The host converts x to fp16 and repacks each core's shard so
that SBUF partition p's entire KB-unit block (KB d-pairs x 2 d-slices
x rows 2p,2p+1) is one contiguous 8 KiB DRAM run. fp16 input halves
the dominant DMA-read traffic (device is HBM/power-bandwidth bound at
~300 GB/s/core) and the fat runs keep the HW DGE descriptor count low
(the queue processes ~1 descriptor per ~8 ns). Total device traffic:
16.8 MB in + 16.8 MB out per core.

Per-core compute, per d-pair unit:
  H stage: TensorE float32r matmuls against +-2^-1.5 * I (1 cyc/row)
  evac:    ScalarE PSUM->SBUF fp16 copy that also de-interleaves
           even/odd w columns, so later DVE ops see packed fp16
  W stage: DVE fp16 tensor_add/sub, packed last dim -> 2x mode
  D stage: DVE fp16 tensor_add/sub on the two d-slices of the pair
Output is written fp16 (tolerance is 2e-2; fp16 error ~1e-3), halving
write traffic; host upcasts to f32. Output DMAs are split between the
GpSimd software DGE queue (bands 0-3) and the Sync HW queue (bands
4-7, issued one block late so they never stall input prefetch).
"""
import sys

sys.path.insert(0, "/opt/trn_rl_repo")

import numpy as np

N, C, D, H, W = 4, 4, 64, 256, 256
NCORES = 8
G_PER_CORE = (N * C) // NCORES        # 2
KP = D // 2                           # 32 d-pairs per g
KB = 4                                # units per block (staging + DMA)
NBLK = G_PER_CORE * (KP // KB)        # 16 blocks per core
S3 = float(2.0 ** -1.5)

IN_BUFS = 5
WE_BUFS = 6
WT_BUFS = 6
OS_BUFS = 4
PSUM_BUFS = 4

_CACHE = {}


def _build_filter_lhst():
    """Stationary operands: +S3*I and -S3*I, as [2, 128, 128] fp16."""
    eye = np.eye(128, dtype=np.float16)
    return np.stack([np.float16(S3) * eye, np.float16(-S3) * eye])


def _repack(xg):
    """[G, D, H, W] f32 -> [NBLK, 128, KB*1024] fp16 where block tile
    partition p holds, for each unit k4, rows 2p,2p+1 of both d-slices
    (8 KiB contiguous per (block, partition))."""
    x6 = xg.astype(np.float16).reshape(
        G_PER_CORE, KP // KB, KB, 2, 128, 2, W)
    # [g, kb, k4, s, p, r, w] -> [g, kb, p, k4, s, r, w]
    xr = x6.transpose(0, 1, 4, 2, 3, 5, 6)
    return np.ascontiguousarray(
        xr.reshape(NBLK, 128, KB * 1024))


def _build_nc():
    import concourse.bass as bass
    import concourse.tile as tile
    from concourse import bacc, mybir

    f32 = mybir.dt.float32
    f32r = mybir.dt.float32r
    f16 = mybir.dt.float16
    nc = bacc.Bacc(None)
    x_d = nc.declare_dram_parameter("x", [NBLK, 128, KB * 1024], f16,
                                    isOutput=False)
    ft_d = nc.declare_dram_parameter("ft", [2, 128, 128], f16,
                                     isOutput=False)
    # h'-major fp16 layout: per (band, g, partition=h') a k-block of KB
    # is one contiguous 1 KiB run in DRAM (host transposes k, h' back)
    o_d = nc.declare_dram_parameter("out", [8, G_PER_CORE, 128, KP, 128],
                                    f16, isOutput=True)

    with tile.TileContext(nc) as tc:
        with (
            tc.tile_pool(name="cst", bufs=1) as cst,
            tc.tile_pool(name="inp", bufs=IN_BUFS) as inp,
            tc.tile_pool(name="we", bufs=WE_BUFS) as wep,
            tc.tile_pool(name="wt", bufs=WT_BUFS) as wtp,
            tc.tile_pool(name="os", bufs=OS_BUFS) as osp,
            tc.tile_pool(name="ps", bufs=PSUM_BUFS, space="PSUM") as psp,
        ):
            ft = cst.tile([128, 256], f16, tag="ft")
            nc.sync.dma_start(
                ft.rearrange("p (i c) -> p i c", i=2),
                ft_d.rearrange("i p c -> p i c"))
            pos_i = ft[:, 0:128]    # +S3 * I
            neg_i = ft[:, 128:256]  # -S3 * I

            bt = [None] * NBLK      # input block tiles
            ot = [None] * NBLK      # output staging tiles

            def in_dma(b):
                t = inp.tile([128, KB * 1024], f16, tag="xin")
                nc.sync.dma_start(t[:], x_d[b])
                bt[b] = t

            def out_dma_trail(b):
                g, kb = divmod(b, KP // KB)
                for bd in range(4, 8):
                    eng = nc.scalar if bd < 6 else nc.sync
                    eng.dma_start(
                        o_d[bd, g, :, kb * KB:(kb + 1) * KB, :],
                        ot[b][:, bd])
                ot[b] = None

            def compute_block(b):
                g, kb = divmod(b, KP // KB)
                os_t = osp.tile([128, 8 * KB * 128], f16, tag="os")
                # [p, band(8), k4(KB), w'(128)]
                ot3 = os_t.rearrange("p (c q w) -> p c q w", c=8, q=KB)
                ot[b] = ot3
                t5 = bt[b].rearrange("p (k s r w) -> p k s r w",
                                     k=KB, s=2, r=2)
                for k4 in range(KB):
                    # --- H stage: fp16 matmuls vs +-S3*I ---
                    pt = psp.tile([128, 1024], f32, tag="ps")
                    lo = pt[:, 0:512].rearrange("p (s w) -> p s w", s=2)
                    hi = pt[:, 512:1024].rearrange(
                        "p (s w) -> p s w", s=2)
                    x0 = t5[:, k4, :, 0, :]
                    x1 = t5[:, k4, :, 1, :]
                    nc.tensor.matmul(lo, pos_i, x0,
                                     start=True, stop=False)
                    nc.tensor.matmul(hi, pos_i, x0,
                                     start=True, stop=False)
                    nc.tensor.matmul(lo, pos_i, x1,
                                     start=False, stop=True)
                    nc.tensor.matmul(hi, neg_i, x1,
                                     start=False, stop=True)

                    # --- PSUM evac on ScalarE: f32 -> fp16, and
                    # de-interleave w parity for packed DVE reads ---
                    # WE: [p, lh(2), s(2), q(2 parity), w'(128)]
                    we_t = wep.tile([128, 1024], f16, tag="we")
                    we5 = we_t.rearrange("p (l s q w) -> p l s q w",
                                         l=2, s=2, q=2)
                    nc.scalar.activation(
                        we_t.rearrange("p (m q w) -> p m q w",
                                       m=4, q=2),
                        pt.rearrange("p (m w q) -> p m q w",
                                     m=4, q=2),
                        mybir.ActivationFunctionType.Copy)

                    # --- W stage on DVE, fp16 2x ---
                    # WT: [p, c(4 = lh*2+wp), s(2), w'(128)]
                    wt_t = wtp.tile([128, 1024], f16, tag="wt")
                    wt4 = wt_t.rearrange("p (c s w) -> p c s w",
                                         c=4, s=2)
                    in0 = we5[:, :, :, 0, :]
                    in1 = we5[:, :, :, 1, :]
                    nc.vector.tensor_add(wt4[:, 0::2], in0, in1)
                    nc.vector.tensor_sub(wt4[:, 1::2], in0, in1)

                    # --- D stage on DVE, fp16 2x ---
                    # band = dp*4 + c  (matches reference order)
                    s0 = wt4[:, :, 0, :]
                    s1 = wt4[:, :, 1, :]
                    nc.vector.tensor_add(ot3[:, 0:4, k4], s0, s1)
                    nc.vector.tensor_sub(ot3[:, 4:8, k4], s0, s1)

                # bands 0-3 on the GpSimd SW queue right away; bands
                # 4-7 go on the Sync HW queue two blocks later (see
                # pipeline below) so the wait never delays inputs.
                for bd in range(4):
                    nc.gpsimd.dma_start(
                        o_d[bd, g, :, kb * KB:(kb + 1) * KB, :],
                        ot3[:, bd])

            # software pipeline: inputs prefetch 2 blocks ahead on the
            # Sync queue; Sync-issued output DMAs trail by 1 block so
            # their DVE-completion waits are always already satisfied.
            in_dma(0)
            in_dma(1)
            for b in range(NBLK):
                compute_block(b)
                if b + 2 < NBLK:
                    in_dma(b + 2)
                if b >= 2:
                    out_dma_trail(b - 2)
            out_dma_trail(NBLK - 2)
            out_dma_trail(NBLK - 1)
    nc.finalize()
    return nc


def _get_nc():
    if "nc" not in _CACHE:
        _CACHE["nc"] = _build_nc()
    return _CACHE["nc"]


def kernel(x, low_0, low_1, low_2, high_0, high_1, high_2):
    from concourse.bass_utils import run_bass_kernel_spmd

    x = np.ascontiguousarray(np.asarray(x, dtype=np.float32))
    ft = _build_filter_lhst()
    xs = x.reshape(N * C, D, H, W)
    in_maps = [
        {"x": _repack(xs[c * G_PER_CORE:(c + 1) * G_PER_CORE]),
         "ft": ft}
        for c in range(NCORES)
    ]
    nc = _get_nc()
    res = run_bass_kernel_spmd(nc, in_maps, list(range(NCORES)))
    full = np.empty((8, N * C, KP, 128, 128), dtype=np.float32)
    for c in range(NCORES):
        full[:, c * G_PER_CORE:(c + 1) * G_PER_CORE] = \
            res.results[c]["out"].transpose(0, 1, 3, 2, 4).astype(
                np.float32)
    full = full.reshape(8, N, C, KP, 128, 128)
    return tuple(full[s] for s in range(8))
